# revision 1
# baseline (speedup 1.0000x reference)
"""Trainium2 Bass kernel for nn_Evolution_26697516712465 (deep-snake GNN).

Self-contained: takes FULL inputs, shards batch across 8 NeuronCores internally
(one image per core; each core runs the snake for the polys of its own image),
returns FULL output [128, 128, 2] fp32.
"""
import numpy as np
import ml_dtypes
from contextlib import ExitStack

import concourse.bass as bass
import concourse.bacc as bacc
import concourse.mybir as mybir
import concourse.tile as tile
from concourse.library_config import mlp as mlp_lib
from concourse.bass_utils import run_bass_kernel_spmd

N_CORES = 8
B, C_IN, H, W = 8, 66, 128, 128
NP, V = 128, 128
RO = 4.0
DIL = (1, 1, 1, 2, 2, 4, 4)
NRES = 7
HW = H * W          # 16384
PADW = W + 2        # 130
PIMG = PADW * PADW  # 16900
PADV = 160          # 16 + 128 + 16 circular pad

f32 = mybir.dt.float32
bf16 = mybir.dt.bfloat16
i16 = mybir.dt.int16
AF = mybir.ActivationFunctionType
ALU = mybir.AluOpType

BF = ml_dtypes.bfloat16


def _bcast(ap_obj, n):
    """Append a step-0 (broadcast) innermost free dim of size n to an AP."""
    return bass.AP(tensor=ap_obj.tensor, offset=ap_obj.offset,
                   ap=[*ap_obj.ap, [0, n]])


def build_nc(P):
    """Build the SPMD Bass program. P = max polys per image."""
    nc = bacc.Bacc("TRN2", target_bir_lowering=False, debug=False)
    NV = P * 128           # corner-gather idx count (multiple of 128)
    ICOLS = NV // 16
    PADQ = -(-P // 4) * 4  # snake poly slots (multiple of 4)
    NQB = PADQ // 4

    # ---------------- inputs ----------------
    d_stack0 = nc.declare_dram_parameter("stack0", [128, PIMG], bf16, isOutput=False)
    d_stack1 = nc.declare_dram_parameter("stack1", [70, PIMG], bf16, isOutput=False)
    d_w1p0 = nc.declare_dram_parameter("w1p0", [128, 3, 2, 128], bf16, isOutput=False)
    d_w1p1 = nc.declare_dram_parameter("w1p1", [70, 3, 2, 128], bf16, isOutput=False)
    d_w2t = nc.declare_dram_parameter("w2t", [128, 2, 64], bf16, isOutput=False)
    d_pb0 = nc.declare_dram_parameter("pb0", [128, 2], f32, isOutput=False)
    d_fusb = nc.declare_dram_parameter("fusb", [128, 2], f32, isOutput=False)
    d_idxc = nc.declare_dram_parameter("idxc", [128, 4, ICOLS], i16, isOutput=False)
    d_wcomp = nc.declare_dram_parameter("wcomp", [128, 4, P], f32, isOutput=False)
    d_b2s = nc.declare_dram_parameter("b2s", [128, P, 64], f32, isOutput=False)
    d_coords = nc.declare_dram_parameter("coords", [128, P, 2], bf16, isOutput=False)
    d_iidx = nc.declare_dram_parameter("iidx", [128, PADQ * PADV // 16], i16, isOutput=False)
    d_base = nc.declare_dram_parameter("base", [128, PADQ, 2], f32, isOutput=False)
    d_headw = nc.declare_dram_parameter("headw", [66, 9, 128], bf16, isOutput=False)
    d_headb = nc.declare_dram_parameter("headb", [128, 3], f32, isOutput=False)
    d_resw = nc.declare_dram_parameter("resw", [128, 63, 128], bf16, isOutput=False)
    d_resb = nc.declare_dram_parameter("resb", [128, 3, 7], f32, isOutput=False)
    d_fusw = nc.declare_dram_parameter("fusw", [128, 8, 256], bf16, isOutput=False)
    d_pw1 = nc.declare_dram_parameter("pw1", [128, 10, 256], bf16, isOutput=False)
    d_pb1 = nc.declare_dram_parameter("pb1", [128, 2], f32, isOutput=False)
    d_pw2 = nc.declare_dram_parameter("pw2", [128, 2, 64], bf16, isOutput=False)
    d_pb2 = nc.declare_dram_parameter("pb2", [64, 1], f32, isOutput=False)
    d_pw3 = nc.declare_dram_parameter("pw3", [64, 2], bf16, isOutput=False)
    d_out = nc.declare_dram_parameter("out", [128, PADQ, 2], f32, isOutput=True)

    feat_dram = nc.dram_tensor("feat_dram", [HW, 64], f32)
    cc_in = nc.dram_tensor("cc_in", [NV, 128], bf16)

    with tile.TileContext(nc, num_cores=N_CORES) as tc, ExitStack() as top:
        wpool = top.enter_context(tc.tile_pool(name="weights", bufs=1))
        w2t_t = wpool.tile([128, 2, 64], bf16)
        nc.sync.dma_start(out=w2t_t, in_=d_w2t[:, :, :])
        pb0_t = wpool.tile([128, 2], f32)
        nc.sync.dma_start(out=pb0_t, in_=d_pb0[:, :])
        fusb_t = wpool.tile([128, 2], f32)
        nc.sync.dma_start(out=fusb_t, in_=d_fusb[:, :])
        idxc_t = wpool.tile([128, 4, ICOLS], i16)
        nc.sync.dma_start(out=idxc_t, in_=d_idxc[:, :, :])
        wcomp_t = wpool.tile([128, 4, P], f32)
        nc.sync.dma_start(out=wcomp_t, in_=d_wcomp[:, :, :])
        b2s_t = wpool.tile([128, P, 64], f32)
        nc.sync.dma_start(out=b2s_t, in_=d_b2s[:, :, :])
        coords_t = wpool.tile([128, P, 2], bf16)
        nc.sync.dma_start(out=coords_t, in_=d_coords[:, :, :])
        iidx_t = wpool.tile([128, PADQ * PADV // 16], i16)
        nc.sync.dma_start(out=iidx_t, in_=d_iidx[:, :])
        base_t = wpool.tile([128, PADQ, 2], f32)
        nc.sync.dma_start(out=base_t, in_=d_base[:, :, :])
        headw_t = wpool.tile([66, 9, 128], bf16)
        headb_t = wpool.tile([128, 3], f32)
        resw_t = wpool.tile([128, 63, 128], bf16)
        resb_t = wpool.tile([128, 3, 7], f32)
        fusw_t = wpool.tile([128, 8, 256], bf16)
        pw1_t = wpool.tile([128, 10, 256], bf16)
        pb1_t = wpool.tile([128, 2], f32)
        pw2_t = wpool.tile([128, 2, 64], bf16)
        pb2_t = wpool.tile([64, 1], f32)
        pw3_t = wpool.tile([64, 2], bf16)

        nc.gpsimd.load_library(mlp_lib)

        # relu1 lives across conv1 + conv2
        with tc.tile_pool(name="relu1", bufs=1) as rpool:
            r1 = [rpool.tile([128, HW], bf16, tag=f"r1_{m}", name=f"r1_{m}")
                  for m in range(2)]

            # ------------ conv1: 3x3 66->256 (bf16, K packed 128+70) ------------
            with tc.tile_pool(name="stacks", bufs=1) as stpool, \
                 tc.tile_pool(name="psumA", bufs=3, space="PSUM") as ppA:
                st0 = stpool.tile([128, PIMG], bf16)
                HALF = 68 * PADW
                nc.sync.dma_start(out=st0[:, :HALF], in_=d_stack0[:, :HALF])
                nc.sync.dma_start(out=st0[:, HALF:], in_=d_stack0[:, HALF:])
                st1 = stpool.tile([70, PIMG], bf16)
                nc.sync.dma_start(out=st1[:, :HALF], in_=d_stack1[:, :HALF])
                nc.sync.dma_start(out=st1[:, HALF:], in_=d_stack1[:, HALF:])
                w1p0_t = stpool.tile([128, 3, 2, 128], bf16)
                nc.sync.dma_start(out=w1p0_t, in_=d_w1p0[:, :, :, :])
                w1p1_t = stpool.tile([70, 3, 2, 128], bf16)
                nc.sync.dma_start(out=w1p1_t, in_=d_w1p1[:, :, :, :])

                for t in range(32):          # hw tiles of 512 = 4 image rows
                    y0 = 4 * t
                    for m in range(2):       # out-channel half
                        ps = ppA.tile([128, 512], f32, tag="psA", name="psA")
                        i = 0
                        for (stk, wt) in ((st0, w1p0_t), (st1, w1p1_t)):
                            for kw in range(3):
                                rhs = bass.AP(tensor=stk.tensor,
                                              offset=stk.offset + y0 * PADW + kw,
                                              ap=[stk.ap[0], [PADW, 4], [1, 128]])
                                nc.tensor.matmul(ps, wt[:, kw, m, :], rhs,
                                                 start=(i == 0), stop=(i == 5))
                                i += 1
                        nc.scalar.activation(r1[m][:, t * 512:(t + 1) * 512], ps,
                                             AF.Relu, bias=pb0_t[:, m:m + 1])

            # ------------ conv2: 1x1 256->64, out [hw, 64] fp32 -> DRAM ------------
            with tc.tile_pool(name="psumB", bufs=2, space="PSUM") as ppB, \
                 tc.tile_pool(name="stage", bufs=3) as spool:
                for g in range(16):
                    ps2 = ppB.tile([128, 512], f32, tag="psB", name="psB")
                    for j in range(8):
                        hw0 = (g * 8 + j) * 128
                        for ch in range(2):
                            nc.tensor.matmul(ps2[:, j * 64:(j + 1) * 64],
                                             r1[ch][:, hw0:hw0 + 128],
                                             w2t_t[:, ch, :],
                                             start=(ch == 0), stop=(ch == 1))
                    stg = spool.tile([128, 512], f32, tag="stage", name="stg")
                    nc.vector.tensor_copy(stg, ps2)
                    dst = bass.AP(tensor=feat_dram, offset=g * 65536,
                                  ap=[[512, 128], [1, 512]])
                    nc.sync.dma_start(out=dst, in_=stg)

        # ------------ bilinear gather + weighted sum + vertex rows ------------
        with tc.tile_pool(name="gpool", bufs=1) as gpool:
            gts = []
            for c in range(4):
                gt = gpool.tile([128, P, 64], f32, tag=f"g{c}", name=f"g{c}")
                src = bass.AP(tensor=feat_dram, offset=0, ap=[[64, HW], [1, 64]])
                nc.gpsimd.dma_gather(gt, src, idxc_t[:, c, :], NV, NV, 64,
                                     single_packet=False)
                gts.append(gt)
            vert = gpool.tile([128, P, 64], f32, tag="vert", name="vert")
            tmp = gpool.tile([128, P, 64], f32, tag="tmp", name="tmp")
            for c in range(4):
                wb = _bcast(wcomp_t[:, c, :], 64)
                if c == 0:
                    nc.vector.tensor_tensor(vert, gts[c], wb, ALU.mult)
                else:
                    nc.vector.tensor_tensor(tmp, gts[c], wb, ALU.mult)
                    nc.vector.tensor_tensor(vert, vert, tmp, ALU.add)
            nc.vector.tensor_tensor(vert, vert, b2s_t, ALU.add)

            contrib = gpool.tile([128, P, 128], bf16, tag="contrib", name="contrib")
            nc.vector.memset(contrib, 0.0)
            nc.vector.tensor_copy(contrib[:, :, 0:64], vert)
            nc.vector.tensor_copy(contrib[:, :, 64:66], coords_t)
            # SBUF [v, q, ch] -> DRAM row q*128+v
            dst = bass.AP(tensor=cc_in, offset=0,
                          ap=[[128, 128], [128 * 128, P], [1, 128]])
            nc.sync.dma_start(out=dst, in_=contrib)

        # snake weights load late (off conv1's critical DMA path)
        nc.sync.dma_start(out=headw_t, in_=d_headw[:, :, :])
        nc.sync.dma_start(out=headb_t, in_=d_headb[:, :])
        nc.sync.dma_start(out=resw_t, in_=d_resw[:, :, :])
        nc.sync.dma_start(out=resb_t, in_=d_resb[:, :, :])
        nc.sync.dma_start(out=fusw_t, in_=d_fusw[:, :, :])
        nc.sync.dma_start(out=pw1_t, in_=d_pw1[:, :, :])
        nc.sync.dma_start(out=pb1_t, in_=d_pb1[:, :])
        nc.sync.dma_start(out=pw2_t, in_=d_pw2[:, :, :])
        nc.sync.dma_start(out=pb2_t, in_=d_pb2[:, :])
        nc.sync.dma_start(out=pw3_t, in_=d_pw3[:, :])

        # ---------------- snake ----------------
        with tc.tile_pool(name="snake", bufs=1) as sn, \
             tc.tile_pool(name="psumS", bufs=4, space="PSUM") as ppS, \
             tc.tile_pool(name="psumT", bufs=2, space="PSUM") as ppT:
            # init transpose-gather directly into circular-padded [ch, poly, 160]
            ipad_raw = sn.tile([128, 1, PADQ * PADV], bf16, tag="ipad", name="ipad")
            ccsrc = bass.AP(tensor=cc_in, offset=0, ap=[[128, NV], [1, 128]])
            nc.gpsimd.dma_gather(ipad_raw, ccsrc, iidx_t[:, :],
                                 PADQ * PADV, PADQ * PADV, 128, transpose=True,
                                 single_packet=False)
            ipad = ipad_raw[:, 0, :].rearrange("p (q k) -> p q k", k=PADV)

            spads = [sn.tile([128, PADQ, PADV], bf16, tag=f"spad{k}", name=f"spad{k}")
                     for k in range(8)]

            def circ_conv(dst_pad, src_pad, src_parts, lhsT_of_tap, bias_ap, gam_ap,
                          bet_ap, dilation, residual):
                for qb in range(NQB):
                    ps = ppS.tile([128, 512], f32, tag="psS", name="psS")
                    for t in range(9):
                        off = qb * 4 * PADV + 16 + (t - 4) * dilation
                        rhs = bass.AP(tensor=src_pad.tensor,
                                      offset=src_pad.offset + off,
                                      ap=[[src_pad.ap[0][0], src_parts],
                                          [PADV, 4], [1, 128]])
                        nc.tensor.matmul(ps, lhsT_of_tap(t), rhs,
                                         start=(t == 0), stop=(t == 8))
                    nc.scalar.activation(
                        dst_pad[:, qb * 4:(qb + 1) * 4, 16:144],
                        ps.rearrange("p (a b) -> p a b", a=4), AF.Relu, bias=bias_ap)
                ctr = dst_pad[:, :, 16:144]
                nc.vector.tensor_scalar(ctr, ctr, gam_ap, bet_ap,
                                        op0=ALU.mult, op1=ALU.add)
                if residual is not None:
                    nc.vector.tensor_tensor(ctr, ctr, residual[:, :, 16:144], ALU.add)
                nc.vector.tensor_copy(dst_pad[:, :, 0:16], dst_pad[:, :, 128:144])
                nc.vector.tensor_copy(dst_pad[:, :, 144:160], dst_pad[:, :, 16:32])

            circ_conv(spads[0], ipad[0:66], 66,
                      lambda t: headw_t[:, t, :],
                      headb_t[:, 0:1], headb_t[:, 1:2], headb_t[:, 2:3], 1, None)
            for i in range(NRES):
                circ_conv(spads[i + 1], spads[i], 128,
                          lambda t, i=i: resw_t[:, i * 9 + t, :],
                          resb_t[:, 0, i:i + 1], resb_t[:, 1, i:i + 1],
                          resb_t[:, 2, i:i + 1], DIL[i], spads[i])

            # fusion 1x1 (1024->256) + per-poly max over V (+ fus bias)
            gmax = [sn.tile([128, PADQ], f32, tag=f"gmax{m}", name=f"gmax{m}")
                    for m in range(2)]
            gb = [sn.tile([128, PADQ], bf16, tag=f"gb{m}", name=f"gb{m}")
                  for m in range(2)]
            for m in range(2):
                for qb in range(NQB):
                    ps = ppS.tile([128, 512], f32, tag="psS", name="psS")
                    for k in range(8):
                        sp = spads[k]
                        rhs = bass.AP(tensor=sp.tensor,
                                      offset=sp.offset + qb * 4 * PADV + 16,
                                      ap=[sp.ap[0], [PADV, 4], [1, 128]])
                        nc.tensor.matmul(ps, fusw_t[:, k, m * 128:(m + 1) * 128], rhs,
                                         start=(k == 0), stop=(k == 7))
                    nc.vector.tensor_reduce(gmax[m][:, qb * 4:(qb + 1) * 4],
                                            ps.rearrange("p (a b) -> p a b", a=4),
                                            axis=mybir.AxisListType.X, op=ALU.max)
                nc.vector.tensor_scalar(gb[m], gmax[m], fusb_t[:, m:m + 1], None,
                                        op0=ALU.add)

            # pred1: 1280 -> 256, relu
            h1 = [sn.tile([128, PADQ * 128], bf16, tag=f"h1_{m}", name=f"h1_{m}")
                  for m in range(2)]
            for m in range(2):
                for qb in range(NQB):
                    ps = ppS.tile([128, 512], f32, tag="psS", name="psS")
                    for k in range(10):
                        if k < 2:
                            rhs = _bcast(gb[k][:, qb * 4:(qb + 1) * 4], 128)
                        else:
                            sp = spads[k - 2]
                            rhs = bass.AP(tensor=sp.tensor,
                                          offset=sp.offset + qb * 4 * PADV + 16,
                                          ap=[sp.ap[0], [PADV, 4], [1, 128]])
                        nc.tensor.matmul(ps, pw1_t[:, k, m * 128:(m + 1) * 128], rhs,
                                         start=(k == 0), stop=(k == 9))
                    nc.scalar.activation(h1[m][:, qb * 512:(qb + 1) * 512], ps,
                                         AF.Relu, bias=pb1_t[:, m:m + 1])

            # pred2: 256 -> 64, relu
            h2 = sn.tile([64, PADQ * 128], bf16, tag="h2", name="h2")
            for qb in range(NQB):
                ps = ppT.tile([64, 512], f32, tag="psT", name="psT")
                for k in range(2):
                    nc.tensor.matmul(ps, pw2_t[:, k, :],
                                     h1[k][:, qb * 512:(qb + 1) * 512],
                                     start=(k == 0), stop=(k == 1))
                nc.scalar.activation(h2[:, qb * 512:(qb + 1) * 512], ps, AF.Relu,
                                     bias=pb2_t[:, 0:1])

            # pred3: 64 -> 2 per poly -> [128 v, PADQ, 2]
            ps3 = ppT.tile([128, PADQ * 2], f32, tag="psT3", name="psT3", bufs=1)
            for j in range(PADQ):
                nc.tensor.matmul(ps3[:, j * 2:(j + 1) * 2],
                                 h2[:, j * 128:(j + 1) * 128], pw3_t[:, :],
                                 start=True, stop=True)
            o_t = sn.tile([128, PADQ, 2], f32, tag="o_t", name="o_t")
            nc.vector.tensor_tensor(o_t, ps3.rearrange("p (a b) -> p a b", b=2),
                                    base_t, ALU.add)
            nc.sync.dma_start(out=d_out[:, :, :], in_=o_t)

    nc.compile()
    return nc


_NC_CACHE = {}


def _get_nc(P):
    if P not in _NC_CACHE:
        _NC_CACHE[P] = build_nc(P)
    return _NC_CACHE[P]


def _host_prep(inputs, P, counts, order, offs):
    """Build per-core in_maps."""
    cnn = np.asarray(inputs["cnn_feature"], np.float32)
    ipoly = np.asarray(inputs["i_it_poly"], np.float32)
    cpoly = np.asarray(inputs["c_it_poly"], np.float32)
    w1 = np.asarray(inputs["proj_w1"], np.float32)
    b2 = np.asarray(inputs["proj_b2"], np.float32)
    w2 = np.asarray(inputs["proj_w2"], np.float32)[:, :, 0, 0]  # [64, 256]
    NV = P * 128
    PADQ = -(-P // 4) * 4

    # ---- grid-sample host math (fp32, matches reference) ----
    ix = ipoly[..., 0] - np.float32(0.5)
    iy = ipoly[..., 1] - np.float32(0.5)
    x0 = np.floor(ix); y0 = np.floor(iy)
    wx = (ix - x0).astype(np.float32); wy = (iy - y0).astype(np.float32)
    x0i = x0.astype(np.int64); y0i = y0.astype(np.int64)
    corner_r = []; corner_w = []
    for dy, dx in ((0, 0), (0, 1), (1, 0), (1, 1)):
        xi = x0i + dx; yi = y0i + dy
        valid = (xi >= 0) & (xi < W) & (yi >= 0) & (yi < H)
        xc = np.clip(xi, 0, W - 1); yc = np.clip(yi, 0, H - 1)
        hw = yc * W + xc
        jt = hw // 128; p = hw % 128
        r = (jt // 8) * 1024 + p * 8 + (jt % 8)      # feat_dram row remap
        wgt = (wx if dx else (1 - wx)) * (wy if dy else (1 - wy))
        corner_r.append(r.astype(np.int64))
        corner_w.append((wgt * valid).astype(np.float32))
    s_v = np.sum(corner_w, axis=0)                    # [NP, V]

    # ---- shared packed weights ----
    w1p0 = np.zeros((128, 3, 2, 128), np.float32)
    w1p1 = np.zeros((70, 3, 2, 128), np.float32)
    for r0 in range(128):
        kh, ci = (0, r0) if r0 < 66 else (1, r0 - 66)
        for kw in range(3):
            for m in range(2):
                w1p0[r0, kw, m, :] = w1[m * 128:(m + 1) * 128, ci, kh, kw]
    for r1 in range(70):
        kh, ci = (1, 62 + r1) if r1 < 4 else (2, r1 - 4)
        for kw in range(3):
            for m in range(2):
                w1p1[r1, kw, m, :] = w1[m * 128:(m + 1) * 128, ci, kh, kw]
    w2t = np.transpose(w2, (1, 0)).reshape(2, 128, 64).transpose(1, 0, 2)

    headw = np.transpose(np.asarray(inputs["head_w"], np.float32), (1, 2, 0))
    headb = np.stack([np.asarray(inputs["head_b"], np.float32),
                      np.asarray(inputs["head_g"], np.float32),
                      np.asarray(inputs["head_bt"], np.float32)], axis=1)
    resw = np.transpose(np.asarray(inputs["res_w"], np.float32), (2, 0, 3, 1))
    resw = resw.reshape(128, 63, 128)
    resb = np.stack([np.asarray(inputs["res_b"], np.float32).T,
                     np.asarray(inputs["res_g"], np.float32).T,
                     np.asarray(inputs["res_bt"], np.float32).T], axis=1)
    fusw = np.transpose(np.asarray(inputs["fus_w"], np.float32).reshape(256, 8, 128),
                        (2, 1, 0))
    pw1 = np.transpose(np.asarray(inputs["pw1"], np.float32).reshape(256, 10, 128),
                       (2, 1, 0))
    pb1 = np.asarray(inputs["pb1"], np.float32).reshape(2, 128).T
    pw2 = np.transpose(np.asarray(inputs["pw2"], np.float32).reshape(64, 2, 128),
                       (2, 1, 0))
    pb2 = np.asarray(inputs["pb2"], np.float32).reshape(64, 1)
    pw3 = np.asarray(inputs["pw3"], np.float32).T
    pb3 = np.asarray(inputs["pb3"], np.float32)
    pb0 = np.asarray(inputs["proj_b1"], np.float32).reshape(2, 128).T
    fusb = np.asarray(inputs["fus_b"], np.float32).reshape(2, 128).T

    shared = {
        "w1p0": w1p0.astype(BF), "w1p1": w1p1.astype(BF), "w2t": w2t.astype(BF),
        "pb0": pb0, "fusb": fusb,
        "headw": headw.astype(BF), "headb": headb,
        "resw": resw.astype(BF), "resb": resb,
        "fusw": fusw.astype(BF), "pw1": pw1.astype(BF), "pb1": pb1,
        "pw2": pw2.astype(BF), "pb2": pb2, "pw3": pw3.astype(BF),
    }

    def pack16(idx_flat, cols):
        tab = np.zeros((16, cols), np.int16)
        n = len(idx_flat)
        tab[np.arange(n) % 16, np.arange(n) // 16] = idx_flat.astype(np.int16)
        return np.tile(tab, (8, 1))

    ind = np.asarray(inputs["ind"]).astype(np.int64)
    in_maps = []
    for c in range(N_CORES):
        img = cnn[c]
        img_pad = np.zeros((C_IN, PADW, PADW), np.float32)
        img_pad[:, 1:129, 1:129] = img
        flat = img_pad.reshape(C_IN, PIMG)
        stack0 = np.zeros((128, PIMG), np.float32)
        stack1 = np.zeros((70, PIMG), np.float32)
        stack0[0:66] = flat
        stack0[66:128, :PIMG - 130] = flat[0:62, 130:]
        stack1[0:4, :PIMG - 130] = flat[62:66, 130:]
        stack1[4:70, :PIMG - 260] = flat[0:66, 260:]

        own = order[offs[c]:offs[c + 1]]
        nown = len(own)
        idxc = np.zeros((4, NV), np.int64)
        wcomp = np.zeros((128, 4, P), np.float32)
        for cc in range(4):
            idxc[cc, :nown * 128] = corner_r[cc][own].reshape(-1)
            wcomp[:, cc, :nown] = corner_w[cc][own].T
        b2s = np.zeros((128, P, 64), np.float32)
        b2s[:, :nown, :] = s_v[own].T[:, :, None] * b2[None, None, :]
        coords = np.zeros((128, P, 2), np.float32)
        coords[:, :nown, :] = (cpoly[own] * RO).transpose(1, 0, 2)

        iidx = np.zeros(PADQ * PADV, np.int64)
        kk = np.arange(PADV)
        for q in range(nown):
            iidx[q * PADV:(q + 1) * PADV] = q * 128 + (kk + 112) % 128
        base = np.zeros((128, PADQ, 2), np.float32)
        if nown:
            base[:, :nown, :] = (ipoly[own] * RO + pb3[None, None, :]) \
                .transpose(1, 0, 2).astype(np.float32)

        m = {
            "stack0": stack0.astype(BF), "stack1": stack1.astype(BF),
            "idxc": np.stack([pack16(idxc[cc], NV // 16) for cc in range(4)], axis=1),
            "wcomp": wcomp, "b2s": b2s, "coords": coords.astype(BF),
            "iidx": pack16(iidx, PADQ * PADV // 16),
            "base": base,
        }
        m.update(shared)
        in_maps.append(m)
    return in_maps


def kernel(**inputs):
    ind = np.asarray(inputs["ind"]).astype(np.int64)
    counts = np.bincount(ind, minlength=N_CORES)
    P = int(counts.max())
    assert P <= 31, f"per-image poly count {P} exceeds int16 gather range"
    order = np.argsort(ind, kind="stable")
    offs = np.concatenate([[0], np.cumsum(counts)])

    nc = _get_nc(P)
    in_maps = _host_prep(inputs, P, counts, order, offs)
    res = None
    last_err = None
    for _attempt in range(3):
        try:
            res = run_bass_kernel_spmd(nc, in_maps, list(range(N_CORES)))
            break
        except Exception as e:  # rare transient device error; retry
            last_err = e
    if res is None:
        raise last_err

    out = np.zeros((NP, V, 2), np.float32)
    for c in range(N_CORES):
        oc = res.results[c]["out"]  # [128v, PADQ, 2]
        own = order[offs[c]:offs[c + 1]]
        for q, opoly in enumerate(own):
            out[opoly] = oc[:, q, :]
    return out



# revision 2
# speedup vs baseline: 1.0103x; 1.0103x over previous
"""Trainium2 Bass kernel for nn_Evolution_26697516712465 (deep-snake GNN), v2.

fp8e4 DoubleRow matmuls throughout; fused relu+residual on DVE/Pool;
PE-transpose for the snake input layout; chunked conv1 pipeline.
Takes FULL inputs, shards batch across 8 NeuronCores (one image per core),
returns FULL output [128, 128, 2] fp32.
"""
import numpy as np
import ml_dtypes
from contextlib import ExitStack

import concourse.bass as bass
import concourse.bacc as bacc
import concourse.mybir as mybir
import concourse.tile as tile
from concourse.bass_utils import run_bass_kernel_spmd

N_CORES = 8
B, C_IN, H, W = 8, 66, 128, 128
NP, V = 128, 128
RO = 4.0
DIL = (1, 1, 1, 2, 2, 4, 4)
NRES = 7
HW = H * W          # 16384
PADW = W + 2        # 130
PIMG = PADW * PADW  # 16900
PADV = 160          # 16 + 128 + 16 circular pad

f32 = mybir.dt.float32
bf16 = mybir.dt.bfloat16
fp8 = mybir.dt.float8e4
i16 = mybir.dt.int16
AF = mybir.ActivationFunctionType
ALU = mybir.AluOpType
DR = mybir.MatmulPerfMode.DoubleRow

BF = ml_dtypes.bfloat16
F8 = ml_dtypes.float8_e4m3

# physical value scales (powers of two)
S_W1, S_R1, S_W2 = 16.0, 32.0, 16.0
S_FEAT, S_COORD, S_STATE = 128.0, 16.0, 128.0
S_WH = 128.0
S_FW, S_GB = 512.0, 256.0
S_P1, S_H1 = 2048.0, 256.0
S_P2, S_H2 = 8192.0, 1024.0
S_P3 = 65536.0

CHROWS = 32          # image rows per conv chunk
NCHUNK = H // CHROWS


def build_nc(P, zb2=True, zpb2=True):
    nc = bacc.Bacc("TRN2", target_bir_lowering=False, debug=False)
    PADQ = -(-P // 4) * 4
    NQB = PADQ // 4
    NVP = PADQ * 128       # gather idx count (padded with 0-idx)
    ICOLS = NVP // 16
    QV = PADQ * PADV       # per-layer state stride

    # ---------------- dram params ----------------
    CH_W = (CHROWS + 2) * PADW   # stack columns per chunk (with halo)
    d_stack = nc.declare_dram_parameter("stack", [100, 2, PIMG], fp8, isOutput=False)
    d_w1q = nc.declare_dram_parameter("w1q", [100, 3, 2, 2, 128], fp8, isOutput=False)
    d_w2q = nc.declare_dram_parameter("w2q", [128, 2, 64], fp8, isOutput=False)
    d_idxg = nc.declare_dram_parameter("idxg", [128, 2, ICOLS], i16, isOutput=False)
    d_wq = nc.declare_dram_parameter("wq", [128, 2, PADQ, 256], fp8, isOutput=False)
    d_b2s = nc.declare_dram_parameter("b2s", [128, PADQ, 64], bf16, isOutput=False)
    d_ident = nc.declare_dram_parameter("ident", [128, 128], bf16, isOutput=False)
    d_cop = nc.declare_dram_parameter("cop", [3, PADQ, PADV], fp8, isOutput=False)
    d_ones = nc.declare_dram_parameter("ones", [128, PADV], fp8, isOutput=False)
    d_headw = nc.declare_dram_parameter("headw", [67, 5, 2, 128], fp8, isOutput=False)
    d_resw = nc.declare_dram_parameter("resw", [128, 7, 5, 2, 128], fp8, isOutput=False)
    d_fusw = nc.declare_dram_parameter("fusw", [128, 4, 2, 2, 128], fp8, isOutput=False)
    d_pw1 = nc.declare_dram_parameter("pw1", [128, 5, 2, 2, 128], fp8, isOutput=False)
    d_pw2 = nc.declare_dram_parameter("pw2", [128, 2, 64], fp8, isOutput=False)
    d_pw3 = nc.declare_dram_parameter("pw3", [64, 2], fp8, isOutput=False)
    # per-partition scale/bias vectors, f32: col0 = head gamma*S_STATE/S_WH,
    # col1/2 = gb bias (m0/m1), col3 = pb1 bias m0, col4 = pb1 bias m1
    d_vecs = nc.declare_dram_parameter("vecs", [128, 5], f32, isOutput=False)
    d_pb2v = nc.declare_dram_parameter("pb2v", [64, 1], f32, isOutput=False)
    d_base = nc.declare_dram_parameter("base", [128, PADQ, 2], f32, isOutput=False)
    d_out = nc.declare_dram_parameter("out", [128, PADQ, 2], f32, isOutput=True)

    # [HW*64] elems as 8192 rows of 128, plus one pad row (gather overhang)
    feat_dram = nc.dram_tensor("feat_dram", [8193, 128], bf16)

    with tile.TileContext(nc, num_cores=N_CORES) as tc, ExitStack() as top:
        wpool = top.enter_context(tc.tile_pool(name="wpool", bufs=1))
        # conv weights first (tiny), then stack chunks, then the rest: the
        # DMA device serializes transfers, so order = criticality.
        w1q_t = wpool.tile([100, 3, 2, 2, 128], fp8)
        nc.sync.dma_start(out=w1q_t, in_=d_w1q[:, :, :, :, :])
        w2q_t = wpool.tile([128, 2, 64], fp8)
        nc.sync.dma_start(out=w2q_t, in_=d_w2q[:, :, :])

        sts = []
        for ch in range(NCHUNK):
            st = wpool.tile([100, 2, CH_W], fp8, tag=f"st{ch}", name=f"st{ch}")
            y0 = ch * CHROWS
            if ch == 0:     # split first chunk so PE starts sooner
                nc.sync.dma_start(
                    out=st[:, :, 0:10 * PADW],
                    in_=d_stack[:, :, 0:10 * PADW])
                nc.sync.dma_start(
                    out=st[:, :, 10 * PADW:22 * PADW],
                    in_=d_stack[:, :, 10 * PADW:22 * PADW])
                nc.sync.dma_start(
                    out=st[:, :, 22 * PADW:CH_W],
                    in_=d_stack[:, :, 22 * PADW:CH_W])
            else:
                nc.sync.dma_start(
                    out=st, in_=d_stack[:, :, y0 * PADW:y0 * PADW + CH_W])
            sts.append(st)

        headw_t = wpool.tile([67, 5, 2, 128], fp8)
        nc.sync.dma_start(out=headw_t, in_=d_headw[:, :, :, :])
        resw_t = wpool.tile([128, 7, 5, 2, 128], fp8)
        nc.sync.dma_start(out=resw_t, in_=d_resw[:, :, :, :, :])

        idxg_t = wpool.tile([128, 2, ICOLS], i16)
        nc.sync.dma_start(out=idxg_t, in_=d_idxg[:, :, :])
        wq_t = wpool.tile([128, 2, PADQ, 256], fp8)
        nc.sync.dma_start(out=wq_t, in_=d_wq[:, :, :, :])
        b2s_t = None
        if not zb2:
            b2s_t = wpool.tile([128, PADQ, 64], bf16)
            nc.sync.dma_start(out=b2s_t, in_=d_b2s[:, :, :])
        ident_t = wpool.tile([128, 128], bf16)
        nc.sync.dma_start(out=ident_t, in_=d_ident[:, :])
        vecs_t = wpool.tile([128, 5], f32)
        nc.sync.dma_start(out=vecs_t, in_=d_vecs[:, :])
        pb2v_t = wpool.tile([64, 1], f32)
        nc.sync.dma_start(out=pb2v_t, in_=d_pb2v[:, :])
        base_t = wpool.tile([128, PADQ, 2], f32)
        nc.sync.dma_start(out=base_t, in_=d_base[:, :, :])

        # snake input [67, PADQ, PADV]: rows 0..63 feat, 64/65 coords, 66 ones
        ipad_t = wpool.tile([67, PADQ, PADV], fp8)
        nc.sync.dma_start(out=ipad_t[64:67, :, :], in_=d_cop[:, :, :])
        # states [128, PADV + 8*QV]; leading PADV = ones slab (bias slot).
        # Ones sit at the LOW end so the dr4 slot-pair AP extent only spans
        # already-written layers (no false WAR/RAW serialization).
        states_t = wpool.tile([128, PADV + 8 * QV], fp8)
        nc.sync.dma_start(out=states_t[:, 0:PADV], in_=d_ones[:, :])

        fusw_t = wpool.tile([128, 4, 2, 2, 128], fp8)
        pw1_t = wpool.tile([128, 5, 2, 2, 128], fp8)
        pw2_t = wpool.tile([128, 2, 64], fp8)
        pw3_t = wpool.tile([64, 2], fp8)

        # preload the Relu act table during the initial DMA wait
        scrap = wpool.tile([1, 4], f32)
        nc.vector.memset(scrap, 0.0)
        nc.scalar.activation(scrap, scrap, AF.Relu)

        # zero the feat_dram pad row (gather overhang reads it with 0 weights)
        zpad = wpool.tile([1, 128], bf16)
        nc.vector.memset(zpad, 0.0)
        nc.sync.dma_start(
            out=bass.AP(tensor=feat_dram, offset=8192 * 128,
                        ap=[[128, 1], [1, 128]]),
            in_=zpad)

        # conv evict engines, balanced by per-op cost
        engs = [nc.scalar, nc.vector, nc.gpsimd, nc.scalar,
                nc.vector, nc.gpsimd, nc.scalar, nc.vector]

        # ---------------- conv1 + conv2 + feat write (chunked) ----------------
        with tc.tile_pool(name="r1p", bufs=2) as r1pool, \
             tc.tile_pool(name="stg", bufs=16) as sgpool, \
             tc.tile_pool(name="psA", bufs=3, space="PSUM") as ppA, \
             tc.tile_pool(name="psB", bufs=2, space="PSUM") as ppB:
            ei = 0
            for ch in range(NCHUNK):
                st = sts[ch]
                r1 = r1pool.tile([128, 2, CHROWS * 128], fp8, tag="r1", name="r1")
                for t in range(CHROWS // 4):        # hw tiles of 512
                    for m in range(2):
                        ps = ppA.tile([128, 512], f32, tag="psA", name="psA")
                        for r in range(4):          # image row within tile
                            for dr in range(3):     # kw offsets
                                rhs = bass.AP(
                                    tensor=st.tensor,
                                    offset=st.offset + (4 * t + r) * PADW + dr,
                                    ap=[st.ap[0], [CH_W, 2], [1, 128]])
                                nc.tensor.matmul(
                                    ps[:, r * 128:(r + 1) * 128],
                                    w1q_t[:, dr, :, m, :], rhs,
                                    start=(dr == 0), stop=(dr == 2),
                                    perf_mode=DR)
                        dst = r1[:, m, t * 512:(t + 1) * 512]
                        e = engs[ei % 8]; ei += 1
                        if e is nc.scalar:
                            e.activation(dst, ps, AF.Relu, scale=S_R1 / S_W1)
                        else:
                            e.tensor_scalar(dst, ps, S_R1 / S_W1, 0.0,
                                            op0=ALU.mult, op1=ALU.max)
                # conv2 on this chunk: 4 psum groups of 1024 px
                for g in range(4):
                    ps2 = ppB.tile([128, 512], f32, tag="psB", name="psB")
                    for j in range(8):
                        px0 = g * 1024 + j * 128
                        rhs = w2q_t[:, :, :]
                        lhsT = bass.AP(tensor=r1.tensor,
                                       offset=r1.offset + px0,
                                       ap=[r1.ap[0], [CHROWS * 128, 2],
                                           [1, 128]])
                        nc.tensor.matmul(ps2[:, j * 64:(j + 1) * 64], lhsT, rhs,
                                         start=True, stop=True, perf_mode=DR)
                    stg = sgpool.tile([128, 512], bf16, tag="stg", name="stg")
                    e = engs[ei % 8]; ei += 1
                    if e is nc.scalar:
                        e.activation(stg, ps2, AF.Copy, scale=S_FEAT / (S_W2 * S_R1))
                    else:
                        e.tensor_scalar(stg, ps2, S_FEAT / (S_W2 * S_R1), None,
                                        op0=ALU.mult)
                    # x-major feat: row (x*128+y), partition = x, 8 y's contig
                    gg = ch * 4 + g
                    dst = bass.AP(tensor=feat_dram, offset=gg * 512,
                                  ap=[[8192, 128], [1, 512]])
                    nc.sync.dma_start(out=dst, in_=stg)

        # ---------------- gather + weighted sum + transpose ----------------
        with tc.tile_pool(name="gp", bufs=1) as gpool, \
             tc.tile_pool(name="psT", bufs=3, space="PSUM") as ppT:
            # feat rows of 128 elems (256B): row k = x*64 + y//2; one gather
            # per x-column (x0 and x0+1), elem 256 = y-window of 4
            src = bass.AP(tensor=feat_dram, offset=0, ap=[[128, 8192], [1, 256]])
            # halve the polys: 4 gathers, ordered so half 0 lands first; the
            # weighted sum (DVE/Pool split) pipelines against the transfers
            NQH = PADQ // 2
            gqs = {}
            for ht in range(2):
                for c in range(2):
                    gq = gpool.tile([128, NQH, 256], bf16,
                                    tag=f"gq{c}_{ht}", name=f"gq{c}_{ht}")
                    idxs = idxg_t[:, c, ht * (ICOLS // 2):(ht + 1) * (ICOLS // 2)]
                    nc.gpsimd.dma_gather(gq, src, idxs, NQH * 128, NQH * 128,
                                         256, elem_step=128, single_packet=False)
                    gqs[(c, ht)] = gq
            vert = gpool.tile([128, PADQ, 64], bf16, tag="vert", name="vert")
            NSP = NQH - NQH // 3       # DVE share of the (1x) fp8-weight mults
            for ht in range(2):
                sl = slice(ht * NQH, (ht + 1) * NQH)
                mms = []
                for c in range(2):
                    mm = gpool.tile([128, NQH, 256], bf16,
                                    tag=f"m{c}_{ht}", name=f"m{c}_{ht}")
                    nc.vector.tensor_tensor(mm[:, 0:NSP, :],
                                            gqs[(c, ht)][:, 0:NSP, :],
                                            wq_t[:, c, ht * NQH:ht * NQH + NSP, :],
                                            ALU.mult)
                    nc.gpsimd.tensor_tensor(mm[:, NSP:NQH, :],
                                            gqs[(c, ht)][:, NSP:NQH, :],
                                            wq_t[:, c, ht * NQH + NSP:
                                                 (ht + 1) * NQH, :],
                                            ALU.mult)
                    mms.append(mm)
                s8 = gpool.tile([128, NQH, 256], bf16, tag=f"s8{ht}", name=f"s8{ht}")
                nc.vector.tensor_tensor(s8, mms[0], mms[1], ALU.add)
                s4 = gpool.tile([128, NQH, 128], bf16, tag=f"s4{ht}", name=f"s4{ht}")
                nc.gpsimd.tensor_tensor(s4, s8[:, :, 0:128], s8[:, :, 128:256],
                                        ALU.add)
                if zb2:
                    nc.vector.tensor_tensor(vert[:, sl, :], s4[:, :, 0:64],
                                            s4[:, :, 64:128], ALU.add)
                else:
                    s2 = gpool.tile([128, NQH, 64], bf16,
                                    tag=f"s2{ht}", name=f"s2{ht}")
                    nc.vector.tensor_tensor(s2, s4[:, :, 0:64], s4[:, :, 64:128],
                                            ALU.add)
                    nc.gpsimd.tensor_tensor(vert[:, sl, :], s2, b2s_t[:, sl, :],
                                            ALU.add)

            # late snake weights: Pool-queue DMAs issue after the gathers,
            # so their transfers can't preempt feat writes or gathers
            nc.gpsimd.dma_start(out=fusw_t, in_=d_fusw[:, :, :, :, :])
            nc.gpsimd.dma_start(out=pw1_t, in_=d_pw1[:, :, :, :, :])
            nc.gpsimd.dma_start(out=pw2_t, in_=d_pw2[:, :, :])
            nc.gpsimd.dma_start(out=pw3_t, in_=d_pw3[:, :])

            # transpose per poly into ipad rows 0..63 (+ circular pads)
            for qb in range(NQB):
                pst = ppT.tile([64, 4, 128], bf16, tag="psT", name="psT")
                for q in range(4):
                    nc.tensor.matmul(pst[:, q, :], vert[:, qb * 4 + q, :],
                                     ident_t, is_transpose=True,
                                     start=True, stop=True)
                e = nc.scalar if qb % 2 == 0 else nc.vector
                if e is nc.scalar:
                    e.activation(ipad_t[0:64, qb * 4:qb * 4 + 4, 16:144], pst,
                                 AF.Copy)
                    e.activation(ipad_t[0:64, qb * 4:qb * 4 + 4, 0:16],
                                 pst[:, :, 112:128], AF.Copy)
                    e.activation(ipad_t[0:64, qb * 4:qb * 4 + 4, 144:160],
                                 pst[:, :, 0:16], AF.Copy)
                else:
                    e.tensor_copy(ipad_t[0:64, qb * 4:qb * 4 + 4, 16:144], pst)
                    e.tensor_copy(ipad_t[0:64, qb * 4:qb * 4 + 4, 0:16],
                                  pst[:, :, 112:128])
                    e.tensor_copy(ipad_t[0:64, qb * 4:qb * 4 + 4, 144:160],
                                  pst[:, :, 0:16])

        # ---------------- snake ----------------
        with tc.tile_pool(name="sn", bufs=1) as sn, \
             tc.tile_pool(name="psS", bufs=4, space="PSUM") as ppS, \
             tc.tile_pool(name="psU", bufs=2, space="PSUM") as ppU:

            def circ_conv(l, src_t, src_off, src_parts, w_t, dilation):
                """layer l: reads src (states slice or ipad), writes states[:,l]."""
                dst_off = l * QV
                for qb in range(NQB):
                    ps = ppS.tile([128, 512], f32, tag="psS", name="psS")
                    for q in range(4):
                        poff = src_off + (qb * 4 + q) * PADV + 16
                        po = ps[:, q * 128:(q + 1) * 128]
                        for dr in range(4):
                            off = poff + (2 * dr - 4) * dilation
                            rhs = bass.AP(tensor=src_t.tensor,
                                          offset=src_t.offset + off,
                                          ap=[[src_t.ap[0][0], src_parts],
                                              [dilation, 2], [1, 128]])
                            nc.tensor.matmul(po, w_t[:, dr, :, :], rhs,
                                             start=(dr == 0), stop=False,
                                             perf_mode=DR)
                        # dr4: for l>0: slot0 = ones/bias (low offset),
                        # slot1 = tap8 -- extent spans only old layers
                        off8 = poff + 4 * dilation
                        if l == 0:
                            rhs = bass.AP(tensor=src_t.tensor,
                                          offset=src_t.offset + off8,
                                          ap=[[src_t.ap[0][0], src_parts],
                                              [0, 2], [1, 128]])
                        else:
                            rhs = bass.AP(tensor=src_t.tensor,
                                          offset=src_t.offset + 16,
                                          ap=[[src_t.ap[0][0], src_parts],
                                              [off8 - 16, 2], [1, 128]])
                        nc.tensor.matmul(po, w_t[:, 4, :, :], rhs,
                                         start=False, stop=True, perf_mode=DR)
                    dst0 = PADV + l * QV
                    ctr = states_t[:, dst0 + qb * 4 * PADV:].rearrange(
                        "p (q k) -> p q k", k=PADV)[:, 0:4, 16:144]
                    e = nc.vector if qb % 2 == 0 else nc.gpsimd
                    if l == 0:
                        e.tensor_scalar(ctr, ps.rearrange("p (a b) -> p a b", a=4),
                                        vecs_t[:, 0:1], 0.0,
                                        op0=ALU.mult, op1=ALU.max)
                    else:
                        prev = states_t[:, dst0 - QV + qb * 4 * PADV:].rearrange(
                            "p (q k) -> p q k", k=PADV)[:, 0:4, 16:144]
                        e.scalar_tensor_tensor(
                            ctr, ps.rearrange("p (a b) -> p a b", a=4), 0.0,
                            prev, op0=ALU.max, op1=ALU.add)
                    # circular pads per 2 qb (Act), keeps next layer flowing
                    if qb % 2 == 1:
                        qs = (qb - 1) * 4
                        lay = states_t[:, dst0 + qs * PADV:].rearrange(
                            "p (q k) -> p q k", k=PADV)
                        nc.scalar.activation(lay[:, 0:8, 0:16],
                                             lay[:, 0:8, 112:128], AF.Copy)
                        nc.scalar.activation(lay[:, 0:8, 144:160],
                                             lay[:, 0:8, 16:32], AF.Copy)

            circ_conv(0, ipad_t, 0, 67, headw_t, 1)
            for i in range(NRES):
                circ_conv(i + 1, states_t, PADV + i * QV, 128, resw_t[:, i], DIL[i])

            # fusion + max + gb (per-qb so pred1 starts without a barrier)
            gb = sn.tile([128, 2, PADQ], fp8, tag="gb", name="gb")
            gmax = sn.tile([128, 2, PADQ], f32, tag="gmax", name="gmax")
            for qb in range(NQB):
                for m in range(2):
                    ps = ppS.tile([128, 512], f32, tag="psS", name="psS")
                    for q in range(4):
                        for dr in range(4):
                            off = PADV + (2 * dr) * QV \
                                + (qb * 4 + q) * PADV + 16
                            rhs = bass.AP(tensor=states_t.tensor,
                                          offset=states_t.offset + off,
                                          ap=[states_t.ap[0], [QV, 2], [1, 128]])
                            nc.tensor.matmul(ps[:, q * 128:(q + 1) * 128],
                                             fusw_t[:, dr, :, m, :], rhs,
                                             start=(dr == 0), stop=(dr == 3),
                                             perf_mode=DR)
                    nc.vector.tensor_reduce(gmax[:, m, qb * 4:(qb + 1) * 4],
                                            ps.rearrange("p (a b) -> p a b", a=4),
                                            axis=mybir.AxisListType.X, op=ALU.max)
                    nc.vector.tensor_scalar(gb[:, m, qb * 4:(qb + 1) * 4],
                                            gmax[:, m, qb * 4:(qb + 1) * 4],
                                            S_GB / S_FW, vecs_t[:, 1 + m:2 + m],
                                            op0=ALU.mult, op1=ALU.add)

            # pred1 -> h1 (relu), pred2 -> h2 (relu), pred3 -> out
            h1 = sn.tile([128, 2, PADQ * 128], fp8, tag="h1", name="h1")
            h2 = sn.tile([64, PADQ * 128], fp8, tag="h2", name="h2")
            ps3 = ppU.tile([128, PADQ * 2], f32, tag="ps3", name="ps3", bufs=1)
            for qb in range(NQB):
                for m in range(2):
                    ps = ppS.tile([128, 512], f32, tag="psS", name="psS")
                    for q in range(4):
                        po = ps[:, q * 128:(q + 1) * 128]
                        rhs = bass.AP(tensor=gb.tensor,
                                      offset=gb.offset + qb * 4 + q,
                                      ap=[gb.ap[0], [PADQ, 2], [0, 128]])
                        nc.tensor.matmul(po, pw1_t[:, 0, :, m, :], rhs,
                                         start=True, stop=False, perf_mode=DR)
                        for dr in range(4):
                            off = PADV + (2 * dr) * QV \
                                + (qb * 4 + q) * PADV + 16
                            rhs = bass.AP(tensor=states_t.tensor,
                                          offset=states_t.offset + off,
                                          ap=[states_t.ap[0], [QV, 2], [1, 128]])
                            nc.tensor.matmul(po, pw1_t[:, 1 + dr, :, m, :], rhs,
                                             start=False, stop=(dr == 3),
                                             perf_mode=DR)
                    nc.scalar.activation(h1[:, m, qb * 512:(qb + 1) * 512], ps,
                                         AF.Relu, bias=vecs_t[:, 3 + m:4 + m],
                                         scale=S_H1 / S_P1)
                ps2 = ppU.tile([64, 512], f32, tag="psU", name="psU")
                rhs = bass.AP(tensor=h1.tensor, offset=h1.offset + qb * 512,
                              ap=[h1.ap[0], [PADQ * 128, 2], [1, 512]])
                nc.tensor.matmul(ps2, pw2_t[:, :, :], rhs,
                                 start=True, stop=True, perf_mode=DR)
                if zpb2:     # pb2 == 0: relu+scale on Pool, freeing Act
                    nc.gpsimd.tensor_scalar(h2[:, qb * 512:(qb + 1) * 512], ps2,
                                            S_H2 / S_P2, 0.0,
                                            op0=ALU.mult, op1=ALU.max)
                else:
                    nc.scalar.activation(h2[:, qb * 512:(qb + 1) * 512], ps2,
                                         AF.Relu, bias=pb2v_t[:, 0:1],
                                         scale=S_H2 / S_P2)
                for q in range(4):
                    j = qb * 4 + q
                    nc.tensor.matmul(ps3[:, j * 2:(j + 1) * 2],
                                     h2[:, j * 128:(j + 1) * 128], pw3_t[:, :],
                                     start=True, stop=True)
            o_t = sn.tile([128, PADQ, 2], f32, tag="o_t", name="o_t")
            nc.vector.scalar_tensor_tensor(
                o_t, ps3.rearrange("p (a b) -> p a b", b=2), 1.0 / S_P3,
                base_t, op0=ALU.mult, op1=ALU.add)
            nc.sync.dma_start(out=d_out[:, :, :], in_=o_t)

    nc.compile()
    return nc


_NC_CACHE = {}


def _get_nc(P, zb2=True, zpb2=True):
    key = (P, zb2, zpb2)
    if key not in _NC_CACHE:
        _NC_CACHE[key] = build_nc(P, zb2, zpb2)
    return _NC_CACHE[key]


def _host_prep(inputs, P, counts, order, offs):
    PADQ = -(-P // 4) * 4
    NVP = PADQ * 128
    QV = PADQ * PADV
    cnn = np.asarray(inputs["cnn_feature"], np.float32)
    ipoly = np.asarray(inputs["i_it_poly"], np.float32)
    cpoly = np.asarray(inputs["c_it_poly"], np.float32)
    w1 = np.asarray(inputs["proj_w1"], np.float32)
    b1 = np.asarray(inputs["proj_b1"], np.float32)
    w2 = np.asarray(inputs["proj_w2"], np.float32)[:, :, 0, 0]  # [64, 256]
    b2 = np.asarray(inputs["proj_b2"], np.float32)
    head_w = np.asarray(inputs["head_w"], np.float32)    # [128, 66, 9]
    head_b = np.asarray(inputs["head_b"], np.float32)
    head_g = np.asarray(inputs["head_g"], np.float32)
    head_bt = np.asarray(inputs["head_bt"], np.float32)
    res_w = np.asarray(inputs["res_w"], np.float32)      # [7, 128, 128, 9]
    res_b = np.asarray(inputs["res_b"], np.float32)
    res_g = np.asarray(inputs["res_g"], np.float32)
    res_bt = np.asarray(inputs["res_bt"], np.float32)
    fus_w = np.asarray(inputs["fus_w"], np.float32)      # [256, 1024]
    fus_b = np.asarray(inputs["fus_b"], np.float32)
    pw1 = np.asarray(inputs["pw1"], np.float32)          # [256, 1280]
    pb1 = np.asarray(inputs["pb1"], np.float32)
    pw2 = np.asarray(inputs["pw2"], np.float32)          # [64, 256]
    pb2 = np.asarray(inputs["pb2"], np.float32)
    pw3 = np.asarray(inputs["pw3"], np.float32)          # [2, 64]
    pb3 = np.asarray(inputs["pb3"], np.float32)

    # ---- grid-sample host math ----
    # feat is x-major: dram row (128 bf16 elems) k = x*64 + y//2.
    # gather c in {0,1} targets column x0+c; elem 256 = y-window [2ky, 2ky+3].
    ix = ipoly[..., 0] - np.float32(0.5)
    iy = ipoly[..., 1] - np.float32(0.5)
    x0 = np.floor(ix); y0 = np.floor(iy)
    wx = (ix - x0).astype(np.float32); wy = (iy - y0).astype(np.float32)
    x0i = x0.astype(np.int64); y0i = y0.astype(np.int64)
    ky = np.clip(y0i, 0, 126) // 2
    gidx = []                                          # [2][NP, V]
    for dx in (0, 1):
        bx = np.clip(x0i + dx, 0, 127)
        gidx.append(bx * 64 + ky)
    # weight of corner (dy,dx) placed at slot y0+dy-2*ky of gather dx
    wslot = np.zeros((2, 4, NP, V), np.float32)        # [gather, slot, n, v]
    s_v = np.zeros((NP, V), np.float32)
    for dy in (0, 1):
        for dx in (0, 1):
            xi = x0i + dx; yi = y0i + dy
            valid = (xi >= 0) & (xi < W) & (yi >= 0) & (yi < H)
            wgt = ((wx if dx else (1 - wx)) * (wy if dy else (1 - wy))
                   * valid).astype(np.float32)
            s_v += wgt
            sy = yi - 2 * ky                           # in [0,3] where valid
            syc = np.clip(sy, 0, 3)
            for s in range(4):
                wslot[dx, s][(syc == s) & valid] = wgt[(syc == s) & valid]

    # ---- conv1 stack rows: 199 = 198 (ci,kh) + ones; slot s row q = idx 100s+q
    def row_of(q, s):
        idx = 100 * s + q
        if idx < 198:
            return (idx % 66, idx // 66)
        return None                       # idx 198 = ones, 199 = pad

    w1q = np.zeros((100, 3, 2, 2, 128), np.float32)
    for s in range(2):
        for q in range(100):
            ck = row_of(q, s)
            if ck is None:
                continue
            ci, kh = ck
            for kw in range(3):
                for m in range(2):
                    w1q[q, kw, s, m, :] = w1[m * 128:(m + 1) * 128, ci, kh, kw] * S_W1
    # ones row (slot1 q=98, idx 198): conv1 bias at kw=0 only
    for m in range(2):
        w1q[98, 0, 1, m, :] = b1[m * 128:(m + 1) * 128] * S_W1

    w2q = np.zeros((128, 2, 64), np.float32)
    for s in range(2):
        w2q[:, s, :] = w2[:, s * 128:(s + 1) * 128].T * S_W2

    # ---- snake weights ----
    # head: input ch scales (feat 64 @S_FEAT, coords 2 @S_COORD)
    s_in = np.concatenate([np.full(64, S_FEAT), np.full(2, S_COORD)])
    hw_s = head_w * (S_WH / s_in[None, :, None])     # [128, 66, 9]
    headw = np.zeros((67, 5, 2, 128), np.float32)
    for dr in range(4):
        for s in range(2):
            headw[0:66, dr, s, :] = hw_s[:, :, 2 * dr + s].T
    headw[0:66, 4, 0, :] = hw_s[:, :, 8].T
    headw[66, 4, 1, :] = head_b * S_WH               # bias row (ones)
    # head evict scale vec: gamma * S_STATE / S_WH
    vec_headg = head_g * (S_STATE / S_WH)

    # beta bookkeeping: states stored as S_STATE*(cur_l - betacum_l)
    betacum = np.zeros((NRES + 1, 128), np.float32)
    betacum[0] = head_bt
    for i in range(NRES):
        betacum[i + 1] = betacum[i] + res_bt[i]

    resw = np.zeros((128, 7, 5, 2, 128), np.float32)
    for i in range(NRES):
        wg = res_w[i] * res_g[i][:, None, None]      # gamma folded
        for dr in range(4):
            for s in range(2):
                resw[:, i, dr, s, :] = wg[:, :, 2 * dr + s].T
        # dr4 slots SWAPPED on device: slot0 = ones/bias, slot1 = tap8
        resw[:, i, 4, 1, :] = wg[:, :, 8].T
        # bias row: S_STATE * gamma * (b + W.(betacum_{i-1}) summed over taps)
        corr = res_b[i] + np.einsum('oit,i->o', res_w[i], betacum[i])
        resw[0, i, 4, 0, :] = S_STATE * res_g[i] * corr

    fusw = np.zeros((128, 4, 2, 2, 128), np.float32)
    fw = fus_w.reshape(256, 8, 128) * (S_FW / S_STATE)
    for dr in range(4):
        for s in range(2):
            for m in range(2):
                fusw[:, dr, s, m, :] = fw[m * 128:(m + 1) * 128, 2 * dr + s, :].T
    # gb bias: S_GB*(fus_b + sum_l fusw_l . betacum_l)
    fcorr = fus_b + np.einsum('olk,lk->o', fus_w.reshape(256, 8, 128),
                              betacum[0:8])
    gbb = S_GB * fcorr                                # [256]

    pw1q = np.zeros((128, 5, 2, 2, 128), np.float32)
    p1 = pw1.reshape(256, 10, 128).copy()
    p1[:, 0:2, :] *= (S_P1 / S_GB)
    p1[:, 2:10, :] *= (S_P1 / S_STATE)
    for m in range(2):
        for s in range(2):
            pw1q[:, 0, s, m, :] = p1[m * 128:(m + 1) * 128, s, :].T
        for dr in range(4):
            for s in range(2):
                pw1q[:, 1 + dr, s, m, :] = \
                    p1[m * 128:(m + 1) * 128, 2 + 2 * dr + s, :].T
    p1corr = pb1 + np.einsum('olk,lk->o', pw1.reshape(256, 10, 128)[:, 2:10, :],
                             betacum[0:8])
    pb1v = S_H1 * p1corr                              # [256]

    pw2q = np.zeros((128, 2, 64), np.float32)
    for s in range(2):
        pw2q[:, s, :] = (pw2[:, s * 128:(s + 1) * 128] * (S_P2 / S_H1)).T
    pb2v = (S_H2 * pb2).reshape(64, 1)

    pw3q = (pw3.T * (S_P3 / S_H2))                    # [64, 2]

    vecs = np.zeros((128, 5), np.float32)
    vecs[:, 0] = vec_headg
    vecs[:, 1] = gbb[0:128]
    vecs[:, 2] = gbb[128:256]
    vecs[:, 3] = pb1v[0:128]
    vecs[:, 4] = pb1v[128:256]

    ident = np.eye(128, dtype=np.float32)

    def pack16(idx_flat, cols):
        tab = np.zeros((16, cols), np.int16)
        n = len(idx_flat)
        tab[np.arange(n) % 16, np.arange(n) // 16] = idx_flat.astype(np.int16)
        return np.tile(tab, (8, 1))

    shared = {
        "w1q": w1q.astype(F8), "w2q": w2q.astype(F8),
        "ident": ident.astype(BF),
        "headw": headw.astype(F8), "resw": resw.astype(F8),
        "fusw": fusw.astype(F8), "pw1": pw1q.astype(F8),
        "pw2": pw2q.astype(F8), "pw3": pw3q.astype(F8),
        "vecs": vecs, "pb2v": pb2v,
        "ones": np.ones((128, PADV), np.float32).astype(F8),
    }

    ind = np.asarray(inputs["ind"]).astype(np.int64)
    in_maps = []
    kk = np.arange(PADV)
    vwrap = (kk - 16) % 128                           # circular vertex index
    for c in range(N_CORES):
        img = cnn[c]
        img_pad = np.zeros((C_IN, PADW, PADW), np.float32)
        img_pad[:, 1:129, 1:129] = img
        flat = img_pad.reshape(C_IN, PIMG)
        stack = np.zeros((100, 2, PIMG), np.float32)
        for s in range(2):
            for q in range(100):
                ck = row_of(q, s)
                if ck is None:
                    continue
                ci, kh = ck
                if kh == 0:
                    stack[q, s, :] = flat[ci]
                else:
                    stack[q, s, :PIMG - 130 * kh] = flat[ci, 130 * kh:]
        stack[98, 1, :] = 1.0                         # ones row (idx 198)

        own = order[offs[c]:offs[c + 1]]
        nown = len(own)
        idxq = np.zeros((2, NVP), np.int64)
        wqa = np.zeros((128, 2, PADQ, 4, 64), np.float32)
        for g in range(2):
            idxq[g, :nown * 128] = gidx[g][own].reshape(-1)
            wqa[:, g, :nown, :, :] = \
                wslot[g][:, own].transpose(2, 1, 0)[:, :, :, None]
        b2s = np.zeros((128, PADQ, 64), np.float32)
        b2s[:, :nown, :] = s_v[own].T[:, :, None] * (b2 * S_FEAT)[None, None, :]

        cop = np.zeros((3, PADQ, PADV), np.float32)
        if nown:
            cpo = cpoly[own] * RO * S_COORD           # [n, V, 2]
            cop[0, :nown, :] = cpo[:, vwrap, 0]
            cop[1, :nown, :] = cpo[:, vwrap, 1]
        cop[2, :, :] = 1.0                            # ones row for head bias

        base = np.zeros((128, PADQ, 2), np.float32)
        if nown:
            base[:, :nown, :] = (ipoly[own] * RO + pb3[None, None, :]) \
                .transpose(1, 0, 2)

        m = {
            "stack": stack.astype(F8),
            "idxg": np.stack([pack16(idxq[g], NVP // 16) for g in range(2)],
                             axis=1),
            "wq": wqa.reshape(128, 2, PADQ, 256).astype(F8),
            "b2s": b2s.astype(BF),
            "cop": cop.astype(F8), "base": base,
        }
        m.update(shared)
        in_maps.append(m)
    return in_maps


def kernel(**inputs):
    ind = np.asarray(inputs["ind"]).astype(np.int64)
    counts = np.bincount(ind, minlength=N_CORES)
    P = int(counts.max())
    order = np.argsort(ind, kind="stable")
    offs = np.concatenate([[0], np.cumsum(counts)])

    zb2 = bool(np.all(np.asarray(inputs["proj_b2"]) == 0))
    zpb2 = bool(np.all(np.asarray(inputs["pb2"]) == 0))
    nc = _get_nc(P, zb2, zpb2)
    in_maps = _host_prep(inputs, P, counts, order, offs)
    res = None
    last_err = None
    for _attempt in range(3):
        try:
            res = run_bass_kernel_spmd(nc, in_maps, list(range(N_CORES)))
            break
        except Exception as e:
            last_err = e
    if res is None:
        raise last_err

    out = np.zeros((NP, V, 2), np.float32)
    for c in range(N_CORES):
        oc = res.results[c]["out"]
        own = order[offs[c]:offs[c + 1]]
        for q, opoly in enumerate(own):
            out[opoly] = oc[:, q, :]
    return out
